# revision 31
# baseline (speedup 1.0000x reference)
"""Trainium2 Bass kernel for nn_BQuantConv1d.

Math (reference):
    sign[k,f,8g+j] = 2*bit_{7-j}(binary[k,f,g]) - 1
    W[f,n]  = sum_k scale[k,f] * sign[k,f,n]          (NF=4096, NX=1024)
    out     = x @ W.T + bias                          (x: (2,2048,1024))

Sharding: NF split across 8 cores (512 features each); x replicated.

Key layout trick: the contraction dim n = 8g+j is chunked by BIT POSITION j
(n mod 8), not by contiguous ranges. The decode diag-matmuls naturally emit
B3_j[g, f] = W.T[8g+j, f] with g on partitions, which is directly the GEMM
moving operand for chunk j. The host supplies x transposed with rows
permuted into (j, g) order so the GEMM stationary tiles line up — no
on-chip transposes or scatter DMAs anywhere.

fp8 DoubleRow GEMM: both GEMM operands are fp8e4m3, so matmuls run in
MatmulPerfMode.DoubleRow (two K-tiles per pass, 2x throughput). To keep
fp8 quantization error out of the result, x and W are each split into
hi + lo fp8 streams (lo = exact residual of hi); out = xh(Wh+Wl) + xl Wh,
dropping the negligible xl*Wl term (~0.1% of out). 12 DoubleRow matmuls
per token tile replace 8 bf16 matmuls: 3072 vs 4096 PE cycles.

Host-side prep (numpy, off the HW clock): x.T cast to fp8 hi/lo + row
permute; binary packed as uint16 in [f-partition, (ftile, k, g)] layout
with two extra fake k-slots of 0xFF; dmat = stacked 128x128 diagonals
diag(scale[k,f]) with the fake slots carrying diag(-C/2) hi/lo, which
folds the W = 2B - C correction into the decode PSUM accumulation.

Per-core device pipeline (PE-bound):
  DVE:  bits_j = (byts & 1<<s) << (14-s)  -- u16 pattern 0x4000 = bf16 2.0
        (one 4x-mode op per plane; bitcast to bf16 is free)
  PE:   psum_j[g, f] = sum_k bits_j[f, (k,g)].T @ diag(scale[k,f])
        (40 diag matmuls per plane, incl the two -C/2 fake slots; the
        bit value 2.0 supplies the 2x of W = 2B - C)
  ACT:  b3h_j = fp8(psum_j)
  DVE:  b3l_j = psum_j - b3h_j -> fp8
  PE:   out_ps[t,f] = sum_p xh[2p..2p+2].T @ b3h/b3l + xl.T @ b3h
        (DoubleRow, fp8e4m3)
  DVE:  out_sb = out_ps + bias -> f32; DMA out.

Scheduling: PE p-state needs ~3us continuous work to reach 2.4 GHz, so
dummy warmup matmuls bridge the input-DMA window. DMA queue entries are
serviced in order per queue: sync queue = byts, dmat (per-ft tiles for
fine-grained deps), then all of x; scalar queue = small consts + output
writes. PSUM must stay <= 7 banks: all 8 allocated slows the cadence.
"""

import sys

sys.path.insert(0, "/opt/trn_rl_repo")

import numpy as np
import concourse.bass as bass
import concourse.mybir as mybir
import concourse.tile as tile
from concourse import bacc
from concourse.bass_utils import run_bass_kernel_spmd

F32 = mybir.dt.float32
BF16 = mybir.dt.bfloat16
U16 = mybir.dt.uint16
FP8 = mybir.dt.float8e4
Alu = mybir.AluOpType
DROW = mybir.MatmulPerfMode.DoubleRow

NCORES = 8
T = 4096  # tokens (2*2048)
NX = 1024
KB = 8  # real bit planes
KB2 = 10  # k-slots incl two fake -C/2 hi/lo slots
G = 128  # packed groups per row (NX/8)
NJ = 8  # bit positions within a packed byte (n mod 8 chunks)
NFL = 512  # features per core (4096/8)
NFT = NFL // 128  # f-tiles per core = 4
TT = T // 128  # token tiles = 32

_CACHED = {}


def _build_nc():
    nc = bacc.Bacc(None, target_bir_lowering=False, debug=False)

    xh_d = nc.dram_tensor("xtph", [NX, T], FP8, kind="ExternalInput")
    xl_d = nc.dram_tensor("xtpl", [NX, T], FP8, kind="ExternalInput")
    byts_d = nc.dram_tensor("byts", [128, NFT * KB2 * G], U16, kind="ExternalInput")
    dmat_d = nc.dram_tensor("dmat", [128, NFT * KB2 * 128], BF16, kind="ExternalInput")
    bias_d = nc.dram_tensor("biasb", [128, NFL], F32, kind="ExternalInput")
    out_d = nc.dram_tensor("out", [T, NFL], F32, kind="ExternalOutput")

    with tile.TileContext(nc) as tc:
        with (
            tc.tile_pool(name="const", bufs=1) as cpool,
            tc.tile_pool(name="bits", bufs=5) as bpool,
            tc.tile_pool(name="out_sb", bufs=8) as opool,
            tc.tile_pool(name="warm_ps", bufs=1, space="PSUM") as wps,
            tc.tile_pool(name="dec_ps", bufs=2, space="PSUM") as dps,
            tc.tile_pool(name="out_ps", bufs=4, space="PSUM") as ops,
        ):
            # ---- input DMAs. DMA queue entries are serviced largely in
            # order per queue: sync queue carries byts + dmat FIRST, then
            # the bulk x streams; scalar queue takes consts + out writes.
            byts = cpool.tile([128, NFT * KB2 * G], U16)
            nc.sync.dma_start(byts, byts_d[:, :])
            FTW = KB2 * 128
            dmat = []
            for ft in range(NFT):
                dft = cpool.tile([128, FTW], BF16, name=f"dmat{ft}")
                nc.sync.dma_start(dft, dmat_d[:, ft * FTW : (ft + 1) * FTW])
                dmat.append(dft)
            biasb = cpool.tile([128, NFL], F32)
            nc.scalar.dma_start(biasb, bias_d[:, :])

            # x hi/lo, transposed+permuted on host: row j*128+g = x[:, 8g+j].
            # Token-block-major so early GEMM tiles land first.
            xh = cpool.tile([128, NJ, T], FP8)
            xl = cpool.tile([128, NJ, T], FP8)
            TBW = 1024
            for tb in range(T // TBW):
                ts_ = slice(tb * TBW, (tb + 1) * TBW)
                for j in range(NJ):
                    rs = slice(j * 128, (j + 1) * 128)
                    nc.sync.dma_start(xh[:, j, ts_], xh_d[rs, ts_])
                    nc.sync.dma_start(xl[:, j, ts_], xl_d[rs, ts_])

            # B3 hi/lo: W.T rows for n = 8g+j in fp8, GEMM moving operands.
            b3h = cpool.tile([128, NJ, NFL], FP8)
            b3l = cpool.tile([128, NJ, NFL], FP8)

            # ---- PE warmup: p-state ramp needs ~3us continuous work; fill
            # the input-DMA window with dummy matmuls (gap-free handoff).
            wtile = cpool.tile([128, 128], BF16)
            nc.vector.memset(wtile, 0.0)
            wpsum = wps.tile([128, 128], F32, name="warm", tag="warm")
            NWARM = 86
            for w in range(NWARM):
                nc.tensor.matmul(
                    wpsum, wtile, wtile, start=(w == 0), stop=(w == NWARM - 1)
                )

            # ---- decode ----
            bits_tiles = {}

            def extract_block(j):
                # (b & 1<<s) << (14-s): bit lands at u16 position 14 =
                # bf16 pattern 0x4000 = 2.0; bitcast is free. Fake k-slots
                # are 0xFF so every plane sees bit=1 there.
                s = 7 - j
                bits_u = bpool.tile(
                    [128, NFT * KB2 * G], U16, name=f"bits{j}", tag="bits"
                )
                nc.vector.tensor_scalar(
                    bits_u, byts, 1 << s, 14 - s,
                    op0=Alu.bitwise_and, op1=Alu.logical_shift_left,
                )
                bits_tiles[j] = bits_u.bitcast(BF16)

            def decode_block(j):
                bits = bits_tiles.pop(j)
                psum_j = dps.tile([128, NFL], F32, name=f"psj{j}", tag="psum_j")
                for ft in range(NFT):
                    blk = slice(ft * 128, (ft + 1) * 128)
                    for k in range(KB2):
                        nc.tensor.matmul(
                            psum_j[:, blk],
                            bits[:, (ft * KB2 + k) * G : (ft * KB2 + k + 1) * G],
                            dmat[ft][:, k * 128 : (k + 1) * 128],
                            start=(k == 0),
                            stop=(k == KB2 - 1),
                        )
                # W.T rows -> fp8 hi (ACT) + exact fp8 residual lo (DVE)
                nc.scalar.copy(b3h[:, j, :], psum_j)
                nc.vector.tensor_tensor(
                    b3l[:, j, :], psum_j, b3h[:, j, :], op=Alu.subtract
                )

            NEXT_AHEAD = 4
            for j in range(NEXT_AHEAD):
                extract_block(j)
            for j in range(NJ):
                if j + NEXT_AHEAD < NJ:
                    extract_block(j + NEXT_AHEAD)
                decode_block(j)

            # ---- GEMM: out = xh.T(Wh+Wl) + xl.T Wh via fp8 DoubleRow
            # matmuls over j-pair K-tiles; xl.T Wl (~0.1%) is dropped.
            # The b3l stream goes last so its evacuations have time.
            streams = [(xh, b3h), (xl, b3h), (xh, b3l)]

            def gemm_block(tt):
                ts_ = slice(tt * 128, (tt + 1) * 128)
                out_ps = ops.tile([128, NFL], F32, name=f"op{tt}", tag="out_ps")
                nmm = len(streams) * (NJ // 2)
                i = 0
                for xs, bs in streams:
                    for p in range(NJ // 2):
                        js = slice(2 * p, 2 * p + 2)
                        nc.tensor.matmul(
                            out_ps,
                            xs[:, js, ts_],
                            bs[:, js, :],
                            start=(i == 0),
                            stop=(i == nmm - 1),
                            perf_mode=DROW,
                        )
                        i += 1
                out_sb = opool.tile([128, NFL], F32, name=f"os{tt}", tag="out_sb")
                nc.vector.tensor_tensor(out_sb, out_ps, biasb, op=Alu.add)
                nc.scalar.dma_start(out_d[ts_, :], out_sb)

            for tt in range(TT):
                gemm_block(tt)

    nc.finalize()
    return nc


def _host_prep(x, binary, scale, bias):
    """Layout/precision host prep: transpose, cast (fp8 hi/lo), permute,
    slice; the only arithmetic is the tiny per-feature scale sums."""
    from ml_dtypes import bfloat16, float8_e4m3fn

    xf = np.ascontiguousarray(x.reshape(-1, x.shape[-1]))  # (T, NX)
    # x.T with rows permuted to (j, g): row j*128+g = x[:, 8g+j]
    xt = np.ascontiguousarray(xf.T)  # (NX, T)
    xtp = np.ascontiguousarray(
        xt.reshape(G, 8, T).transpose(1, 0, 2).reshape(NX, T)
    )
    xtph = xtp.astype(float8_e4m3fn)
    xtpl = (xtp - xtph.astype(np.float32)).astype(float8_e4m3fn)

    scale2 = scale[..., 0] if scale.ndim == 3 else scale  # (KB, NF)
    c_full = scale2.sum(axis=0, dtype=np.float64)  # (NF,)

    per_core = []
    for i in range(NCORES):
        fsl = slice(i * NFL, (i + 1) * NFL)
        b = binary[:, fsl, :]  # (KB, NFL, G)
        # byts[p, ft, k, g] = binary[k, ft*128+p, g]; fake k slots = 0xFF
        byts = np.full((128, NFT, KB2, G), 0xFF, dtype=np.uint16)
        byts[:, :, :KB, :] = (
            b.reshape(KB, NFT, 128, G).transpose(2, 1, 0, 3).astype(np.uint16)
        )
        byts = np.ascontiguousarray(byts.reshape(128, -1))
        sc = scale2[:, fsl].astype(np.float32)  # (KB, NFL)
        # dmat[p, (ft, k), c] = (c == p) * scale[k, ft*128+p]; the fake
        # slots carry diag(-C/2) split hi/lo so W = 2B - C lands in PSUM.
        # (bits arrive as {0, 2.0}, supplying the 2x.)
        negc_half = (-c_full[fsl] / 2.0).astype(np.float32)
        nh_hi = negc_half.astype(bfloat16)
        nh_lo = (negc_half - nh_hi.astype(np.float32)).astype(bfloat16)
        dm = np.zeros((128, NFT, KB2, 128), dtype=np.float32)
        idx = np.arange(128)
        for ft in range(NFT):
            for k in range(KB):
                dm[idx, ft, k, idx] = sc[k, ft * 128 : (ft + 1) * 128]
            dm[idx, ft, KB, idx] = nh_hi[ft * 128 : (ft + 1) * 128].astype(
                np.float32
            )
            dm[idx, ft, KB + 1, idx] = nh_lo[ft * 128 : (ft + 1) * 128].astype(
                np.float32
            )
        dmat = np.ascontiguousarray(dm.reshape(128, -1).astype(bfloat16))
        biasb = np.ascontiguousarray(
            np.broadcast_to(bias[fsl].astype(np.float32)[None, :], (128, NFL))
        )
        per_core.append(
            {
                "xtph": xtph,
                "xtpl": xtpl,
                "byts": byts,
                "dmat": dmat,
                "biasb": biasb,
            }
        )
    return per_core


def _install_ntff_hook():
    """The agent image's antenv lacks axon_hooks; synthesize it so
    run_bass_kernel_spmd(trace=True) can capture NTFF profiles."""
    import types

    if "antenv.axon_hooks" in sys.modules:
        return
    import antenv
    from trn_agent_boot.trn_boot import _ntff_profile_via_ctypes

    mod = types.ModuleType("antenv.axon_hooks")
    state = {"hook": _ntff_profile_via_ctypes("/opt/axon/libaxon_pjrt.so")}
    mod.set_axon_ntff_profile_hook = lambda h: state.__setitem__("hook", h)
    mod.get_axon_ntff_profile_hook = lambda: state["hook"]
    sys.modules["antenv.axon_hooks"] = mod
    antenv.axon_hooks = mod


def kernel(x, binary, scale, bias, _trace=False):
    x = np.ascontiguousarray(np.asarray(x), dtype=np.float32)
    binary = np.ascontiguousarray(np.asarray(binary), dtype=np.int32)
    scale = np.ascontiguousarray(np.asarray(scale), dtype=np.float32)
    bias = np.ascontiguousarray(np.asarray(bias), dtype=np.float32)

    orig_shape = x.shape[:-1] + (binary.shape[1],)

    if "nc" not in _CACHED:
        _CACHED["nc"] = _build_nc()
    nc = _CACHED["nc"]

    in_maps = _host_prep(x, binary, scale, bias)

    kw = {}
    if _trace:
        _install_ntff_hook()
        kw = dict(trace=True, trace_cores=[0])
    res = run_bass_kernel_spmd(nc, in_maps, core_ids=list(range(NCORES)), **kw)
    out = np.concatenate([res.results[i]["out"] for i in range(NCORES)], axis=1)
    if _trace:
        return out.reshape(orig_shape), res
    return out.reshape(orig_shape)


# revision 32
# speedup vs baseline: 1.3366x; 1.3366x over previous
"""Trainium2 Bass kernel for nn_BQuantConv1d.

Math (reference):
    sign[k,f,8g+j] = 2*bit_{7-j}(binary[k,f,g]) - 1
    W[f,n]  = sum_k scale[k,f] * sign[k,f,n]          (NF=4096, NX=1024)
    out     = x @ W.T + bias                          (x: (2,2048,1024))

Sharding: NF split across 8 cores (512 features each); x replicated.

Key layout trick: the contraction dim n = 8g+j is chunked by BIT POSITION j
(n mod 8), not by contiguous ranges. The decode diag-matmuls naturally emit
B3_j[g, f] = W.T[8g+j, f] with g on partitions, which is directly the GEMM
moving operand for chunk j. The host supplies x transposed with rows
permuted into (j, g) order so the GEMM stationary tiles line up — no
on-chip transposes or scatter DMAs anywhere.

Host-side prep (numpy, off the HW clock): x.T cast to bf16 + row permute;
binary packed as uint16 in [f-partition, (ftile, k, g)] layout; D = the
stacked 128x128 diagonals diag(2*scale[k, f]); row-broadcast tiles of
-C[f] = -sum_k scale[k,f] and bias[f].

Per-core device pipeline (PE-bound, ~91 us incl ~10 us fixed framework
preamble/epilogue; throttled runs ~107 us):
  DVE:  bits_j = (byts & 1<<s) << (14-s)  -- u16 pattern 0x4000 = bf16 2.0
        (one 4x-mode op per plane; bitcast to bf16 is free)
  PE:   psum_j[g, f] = sum_k bits_j[f, (k,g)].T @ diag(scale[k,f])
        (32 diag matmuls per plane; PSUM accumulates the k-sum; the
        bit value 2.0 supplies the 2x of W = 2B - C)
  DVE:  B3_j = psum_j + (-C)  -> bf16
  PE:   out_ps[t, f] = sum_j xtp[g, j, t-tile].T @ B3_j[g, f]
  DVE:  out_sb = out_ps + bias -> f32; DMA out.

Scheduling: PE p-state needs ~3us continuous work to reach 2.4 GHz, so
dummy warmup matmuls bridge the input-DMA window. DMA queue entries are
serviced in order per queue: sync queue = byts, dmat (per-ft tiles for
fine-grained deps), then all of x; scalar queue = small consts + output
writes. PE runs one uninterrupted matmul stream: warmup, 256 decode
matmuls, 256 GEMM matmuls (cost floor: 512 cols x 256 + 128 x 256 =
68.3 us at 2.4 GHz). PSUM must stay <= 7 banks: all 8 allocated slows
the matmul cadence ~15%.
"""

import sys

sys.path.insert(0, "/opt/trn_rl_repo")

import numpy as np
import concourse.bass as bass
import concourse.mybir as mybir
import concourse.tile as tile
from concourse import bacc
from concourse.bass_utils import run_bass_kernel_spmd

F32 = mybir.dt.float32
BF16 = mybir.dt.bfloat16
U16 = mybir.dt.uint16
Alu = mybir.AluOpType

NCORES = 8
T = 4096  # tokens (2*2048)
NX = 1024
KB = 8  # bit planes
G = 128  # packed groups per row (NX/8)
NJ = 8  # bit positions within a packed byte (n mod 8 chunks)
NFL = 512  # features per core (4096/8)
NFT = NFL // 128  # f-tiles per core = 4
TT = T // 128  # token tiles = 32

_CACHED = {}


def _build_nc():
    nc = bacc.Bacc(None, target_bir_lowering=False, debug=False)

    xtp_d = nc.dram_tensor("xtp", [NX, T], BF16, kind="ExternalInput")
    byts_d = nc.dram_tensor("byts", [128, NFT * KB * G], U16, kind="ExternalInput")
    dmat_d = nc.dram_tensor("dmat", [128, NFT * KB * 128], BF16, kind="ExternalInput")
    negc_d = nc.dram_tensor("negc", [128, NFL], F32, kind="ExternalInput")
    bias_d = nc.dram_tensor("biasb", [128, NFL], F32, kind="ExternalInput")
    out_d = nc.dram_tensor("out", [T, NFL], F32, kind="ExternalOutput")

    with tile.TileContext(nc) as tc:
        with (
            tc.tile_pool(name="const", bufs=1) as cpool,
            tc.tile_pool(name="bits", bufs=5) as bpool,
            tc.tile_pool(name="out_sb", bufs=8) as opool,
            tc.tile_pool(name="warm_ps", bufs=1, space="PSUM") as wps,
            tc.tile_pool(name="dec_ps", bufs=2, space="PSUM") as dps,
            tc.tile_pool(name="out_ps", bufs=4, space="PSUM") as ops,
        ):
            # ---- input DMAs. DMA queue entries are serviced largely in
            # order per queue, so the sync queue carries the decode-critical
            # loads (byts, dmat) FIRST and the bulk x stream behind them —
            # x cannot steal bandwidth from the decode inputs. The scalar
            # queue (whose DGE starts a few us later) carries the small
            # consts and the output writes.
            byts = cpool.tile([128, NFT * KB * G], U16)
            nc.sync.dma_start(byts, byts_d[:, :])
            # One tile per f-tile: tile-granular dependency tracking means
            # decode's first matmuls only wait for dmat[0], not all of it.
            FTW = KB * 128
            dmat = []
            for ft in range(NFT):
                dft = cpool.tile([128, FTW], BF16, name=f"dmat{ft}")
                nc.sync.dma_start(dft, dmat_d[:, ft * FTW : (ft + 1) * FTW])
                dmat.append(dft)
            negc = cpool.tile([128, NFL], F32)
            nc.scalar.dma_start(negc, negc_d[:, :])
            biasb = cpool.tile([128, NFL], F32)
            nc.scalar.dma_start(biasb, bias_d[:, :])

            # x, transposed+permuted on host: row j*128+g holds x[:, 8g+j].
            # Loaded token-block-major so early GEMM tiles land first.
            xtp = cpool.tile([128, NJ, T], BF16)
            TBW = 1024  # token block width per DMA
            for tb in range(T // TBW):
                for j in range(NJ):
                    nc.sync.dma_start(
                        xtp[:, j, tb * TBW : (tb + 1) * TBW],
                        xtp_d[j * 128 : (j + 1) * 128, tb * TBW : (tb + 1) * TBW],
                    )

            # B3[g, j, f]: W.T rows for n = 8g+j, bf16, GEMM moving operand.
            b3 = cpool.tile([128, NJ, NFL], BF16)

            # ---- PE warmup: the PE p-state ramp needs ~3us of continuous
            # work to reach 2.4 GHz; fill the input-load window with dummy
            # matmuls so decode starts ramped and gap-free.
            wtile = cpool.tile([128, 128], BF16)
            nc.vector.memset(wtile, 0.0)
            wpsum = wps.tile([128, 128], F32, name="warm", tag="warm")
            NWARM = 86
            for w in range(NWARM):
                nc.tensor.matmul(
                    wpsum, wtile, wtile, start=(w == 0), stop=(w == NWARM - 1)
                )

            # ---- decode: per bit position j, extract bits on DVE, then
            # 32 diag matmuls on PE accumulate the k-sum into PSUM.
            bits_tiles = {}

            def extract_block(j):
                # (b & 1<<s) << (14-s) puts the bit at u16 position 14 =
                # bf16 pattern 0x4000 = 2.0; bitcast is free. The 2x is
                # pre-divided out of dmat (host builds diag(scale), and
                # 2*bit*scale is exactly the 2B term of W = 2B - C).
                s = 7 - j
                bits_u = bpool.tile(
                    [128, NFT * KB * G], U16, name=f"bits{j}", tag="bits"
                )
                nc.vector.tensor_scalar(
                    bits_u, byts, 1 << s, 14 - s,
                    op0=Alu.bitwise_and, op1=Alu.logical_shift_left,
                )
                bits_tiles[j] = bits_u.bitcast(BF16)

            def decode_block(j):
                bits = bits_tiles.pop(j)
                psum_j = dps.tile([128, NFL], F32, name=f"psj{j}", tag="psum_j")
                for ft in range(NFT):
                    blk = slice(ft * 128, (ft + 1) * 128)
                    for k in range(KB):
                        nc.tensor.matmul(
                            psum_j[:, blk],
                            bits[:, (ft * KB + k) * G : (ft * KB + k + 1) * G],
                            dmat[ft][:, k * 128 : (k + 1) * 128],
                            start=(k == 0),
                            stop=(k == KB - 1),
                        )
                # B3_j = psum_j - C  (DVE; GPSIMD cannot access PSUM)
                nc.vector.tensor_tensor(b3[:, j, :], psum_j, negc, op=Alu.add)

            # First 4 extracts run back-to-back so the PE decode stream
            # never waits on the DVE FIFO; later extracts interleave with
            # the psum evacuations.
            NEXT_AHEAD = 4
            for j in range(NEXT_AHEAD):
                extract_block(j)
            for j in range(NJ):
                if j + NEXT_AHEAD < NJ:
                    extract_block(j + NEXT_AHEAD)
                decode_block(j)

            # ---- GEMM: out[t, f] = sum_j xtp_j.T @ B3_j  (+bias on evac)
            def gemm_block(tt, halves=1):
                # halves=2 splits the tile into two f-halves so the final
                # evac + out DMA overlap the last matmuls (tail shaving).
                hw_ = NFL // halves
                out_ps = ops.tile([128, NFL], F32, name=f"op{tt}", tag="out_ps")
                for h in range(halves):
                    fs = slice(h * hw_, (h + 1) * hw_)
                    for j in range(NJ):
                        nc.tensor.matmul(
                            out_ps[:, fs],
                            xtp[:, j, tt * 128 : (tt + 1) * 128],
                            b3[:, j, fs],
                            start=(j == 0),
                            stop=(j == NJ - 1),
                        )
                    out_sb = opool.tile(
                        [128, hw_], F32, name=f"os{tt}_{h}",
                        tag="out_sb" if halves == 1 else f"out_sbh{h}",
                    )
                    nc.vector.tensor_tensor(
                        out_sb, out_ps[:, fs], biasb[:, fs], op=Alu.add
                    )
                    nc.scalar.dma_start(
                        out_d[tt * 128 : (tt + 1) * 128, fs], out_sb
                    )

            for tt in range(TT):
                gemm_block(tt)

    nc.finalize()
    return nc


def _host_prep(x, binary, scale, bias):
    """Layout-only host prep: transpose/cast/permute/slice, no math beyond
    the tiny per-feature scale sums (8*4096 adds)."""
    from ml_dtypes import bfloat16

    xf = np.ascontiguousarray(x.reshape(-1, x.shape[-1]))  # (T, NX)
    # x.T with rows permuted to (j, g): row j*128+g = x[:, 8g+j]
    xt = np.ascontiguousarray(xf.T)  # (NX, T)
    xtp = np.ascontiguousarray(
        xt.reshape(G, 8, T).transpose(1, 0, 2).reshape(NX, T).astype(bfloat16)
    )

    scale2 = scale[..., 0] if scale.ndim == 3 else scale  # (KB, NF)
    c_full = scale2.sum(axis=0, dtype=np.float64)  # (NF,)

    per_core = []
    for i in range(NCORES):
        fsl = slice(i * NFL, (i + 1) * NFL)
        b = binary[:, fsl, :]  # (KB, NFL, G)
        # byts[p, ft, k, g] = binary[k, ft*128+p, g]
        byts = np.ascontiguousarray(
            b.reshape(KB, NFT, 128, G).transpose(2, 1, 0, 3).reshape(128, -1)
        ).astype(np.uint16)
        sc = scale2[:, fsl].astype(np.float32)  # (KB, NFL)
        # dmat[p, (ft, k), c] = (c == p) * scale[k, ft*128+p]
        # (bits arrive as {0, 2.0} so the product is the 2B term of W=2B-C)
        dm = np.zeros((128, NFT, KB, 128), dtype=np.float32)
        idx = np.arange(128)
        for ft in range(NFT):
            for k in range(KB):
                dm[idx, ft, k, idx] = sc[k, ft * 128 : (ft + 1) * 128]
        dmat = np.ascontiguousarray(dm.reshape(128, -1).astype(bfloat16))
        negc = np.ascontiguousarray(
            np.broadcast_to(
                -c_full[fsl].astype(np.float32)[None, :], (128, NFL)
            )
        )
        biasb = np.ascontiguousarray(
            np.broadcast_to(bias[fsl].astype(np.float32)[None, :], (128, NFL))
        )
        per_core.append(
            {
                "xtp": xtp,
                "byts": byts,
                "dmat": dmat,
                "negc": negc,
                "biasb": biasb,
            }
        )
    return per_core


def _install_ntff_hook():
    """The agent image's antenv lacks axon_hooks; synthesize it so
    run_bass_kernel_spmd(trace=True) can capture NTFF profiles."""
    import types

    if "antenv.axon_hooks" in sys.modules:
        return
    import antenv
    from trn_agent_boot.trn_boot import _ntff_profile_via_ctypes

    mod = types.ModuleType("antenv.axon_hooks")
    state = {"hook": _ntff_profile_via_ctypes("/opt/axon/libaxon_pjrt.so")}
    mod.set_axon_ntff_profile_hook = lambda h: state.__setitem__("hook", h)
    mod.get_axon_ntff_profile_hook = lambda: state["hook"]
    sys.modules["antenv.axon_hooks"] = mod
    antenv.axon_hooks = mod


def kernel(x, binary, scale, bias, _trace=False):
    x = np.ascontiguousarray(np.asarray(x), dtype=np.float32)
    binary = np.ascontiguousarray(np.asarray(binary), dtype=np.int32)
    scale = np.ascontiguousarray(np.asarray(scale), dtype=np.float32)
    bias = np.ascontiguousarray(np.asarray(bias), dtype=np.float32)

    orig_shape = x.shape[:-1] + (binary.shape[1],)

    if "nc" not in _CACHED:
        _CACHED["nc"] = _build_nc()
    nc = _CACHED["nc"]

    in_maps = _host_prep(x, binary, scale, bias)

    kw = {}
    if _trace:
        _install_ntff_hook()
        kw = dict(trace=True, trace_cores=[0])
    res = run_bass_kernel_spmd(nc, in_maps, core_ids=list(range(NCORES)), **kw)
    out = np.concatenate([res.results[i]["out"] for i in range(NCORES)], axis=1)
    if _trace:
        return out.reshape(orig_shape), res
    return out.reshape(orig_shape)
